# revision 38
# baseline (speedup 1.0000x reference)
"""Multi-head self-attention (B=2, T=2048, C=1024, H=16) on 8 TRN2 NeuronCores.

Sharding: tensor-parallel over heads. Core m owns heads (2m, 2m+1):
  - qkv^T = (Wqkv_shard^T) @ x^T for its 2 heads (contraction-major layouts;
    host pre-transposes x), fp16 operands, fp32 PSUM accumulation
  - causal attention, flash-style with blockwise exp (no max-subtraction:
    scores are O(1) here); the AV stationary packs [v-dims | 64 ones-columns]
    so the softmax denominator comes out broadcast across 64 PSUM partitions
  - causal query-slicing: diagonal-band blocks only stream queries >= key
    block start; the residual 128x128 triangle is masked on DVE
  - v^T -> v (key-major) via XBAR DMA-transpose (16-bit), no PE transposes
  - partial output projection partial_m = values_m @ Wo[rows of heads m],
    DMA'd straight from PSUM (fp32)
Host sums the 8 partials and adds bias bo.
"""

import numpy as np

import concourse.bass as bass
import concourse.bacc as bacc
import concourse.mybir as mybir
import concourse.tile as tile
from concourse.bass_utils import run_bass_kernel_spmd

B, T, C = 2, 2048, 1024
H, HS = 16, 64
N_CORES = 8
HPC = H // N_CORES            # heads per core = 2
R = B * T                      # 4096 rows total
IC_W = 512                     # i-chunk width (query cols per block)
JT_W = 128                     # j-tile width (key rows per block)
N_IC = T // IC_W               # 4 i-chunks per batch
N_JT = T // JT_W               # 16 j-tiles per batch
F32 = mybir.dt.float32
F32R = mybir.dt.float32r
F16 = mybir.dt.float16


def _build(causal: bool, reps: int = 1):
    nc = bacc.Bacc("TRN2", target_bir_lowering=False, debug=False,
                   num_devices=N_CORES)

    xt_d = nc.dram_tensor("xt", [C, R], F16, kind="ExternalInput")
    w3_d = nc.dram_tensor("w3", [C, 3 * 128], F16, kind="ExternalInput")
    b3_d = nc.dram_tensor("b3", [128, 3], F32, kind="ExternalInput")
    wo_d = nc.dram_tensor("wo", [128, C], F16, kind="ExternalInput")
    # cols 0:128 = causal 128x128 tril mask; cols 128:192 = ones
    mo_d = nc.dram_tensor("mskones", [128, JT_W + 64], F16,
                          kind="ExternalInput")
    out_d = nc.dram_tensor("part", [R, C], F16, kind="ExternalOutput")

    with tile.TileContext(nc) as tc:
        with (
            tc.tile_pool(name="const", bufs=1) as cpool,
            tc.tile_pool(name="qkv", bufs=1) as qkvpool,
            tc.tile_pool(name="xt", bufs=4) as xtpool,
            tc.tile_pool(name="pt", bufs=6) as ptpool,
            tc.tile_pool(name="rcb", bufs=2) as rcbpool,
            tc.tile_pool(name="ot", bufs=2) as otpool,
            tc.tile_pool(name="osb", bufs=2) as opool,
            tc.tile_pool(name="ps_mm", bufs=2, space="PSUM") as ps_mm,
            tc.tile_pool(name="ps_s", bufs=2, space="PSUM") as ps_s,
            tc.tile_pool(name="ps_o", bufs=2, space="PSUM") as ps_o,
            tc.tile_pool(name="ps_p", bufs=2, space="PSUM") as ps_p,
        ):
            for rep in range(reps):
                # ---- constants: qSP gets w3/b3 (+x tiles below); qACT the rest
                # w3 and the first x chunk split + interleaved so the first
                # QKV matmuls start as early as possible
                w3_sb = cpool.tile([128, 8 * 384], F16, tag="w3")
                xt00 = xtpool.tile([128, 8 * IC_W], F16, tag="xt",
                                   name=f"xt_{rep}_0_0")
                for wh in range(2):
                    nc.sync.dma_start(
                        w3_sb[:, 1536 * wh:1536 * (wh + 1)]
                            .rearrange("p (ct c) -> p ct c", ct=4),
                        w3_d.ap()[512 * wh:512 * (wh + 1), :]
                            .rearrange("(ct p) c -> p ct c", p=128))
                    nc.sync.dma_start(
                        xt00[:, 2048 * wh:2048 * (wh + 1)]
                            .rearrange("p (ct c) -> p ct c", ct=4),
                        xt_d.ap()[512 * wh:512 * (wh + 1), 0:IC_W]
                            .rearrange("(ct p) c -> p ct c", p=128))
                bias_sb = cpool.tile([128, 3], F32, tag="b3")
                nc.sync.dma_start(bias_sb[:], b3_d.ap()[:])
                wo_sb = cpool.tile([128, C], F16, tag="wo")
                nc.scalar.dma_start(wo_sb[:], wo_d.ap()[:])
                mo_sb = cpool.tile([128, JT_W + 64], F16, tag="mskones")
                nc.scalar.dma_start(mo_sb[:], mo_d.ap()[:])
                msk_sb = mo_sb[:, 0:JT_W]
                ones_sb = mo_sb[:, JT_W:JT_W + 64]

                # persistent qkv^T (transposed layouts, heads packed 2-up)
                qt2b = [qkvpool.tile([128, T], F16, tag=f"qt2_{b_}",
                                     name=f"qt2_{rep}_{b_}")
                        for b_ in range(B)]
                kt2b = [qkvpool.tile([128, T], F16, tag=f"kt2_{b_}",
                                     name=f"kt2_{rep}_{b_}")
                        for b_ in range(B)]
                vt2b = [qkvpool.tile([128, T], F16, tag=f"vt2_{b_}",
                                     name=f"vt2_{rep}_{b_}")
                        for b_ in range(B)]
                # v in key-major layout + ones cols: per (b,h) [128, 16*128]
                # each 128-col block: [0:64) = v dims, [64:128) = ones
                vn_sb = {(b, h): qkvpool.tile([128, N_JT * 128], F16,
                                              tag=f"vn_{b}_{h}",
                                              name=f"vn_{rep}_{b}_{h}")
                         for b in range(B) for h in range(HPC)}

                # ones cols fill [64:128) of every vn 128-col block; on Pool
                # (SBUF->SBUF) so no compute queue ever stalls the first AVs
                for b in range(B):
                    for h in range(HPC):
                        nc.gpsimd.tensor_copy(
                            vn_sb[(b, h)][:]
                                .rearrange("p (jt d) -> p jt d", d=128)
                                [:, :, 64:128],
                            ones_sb.unsqueeze(1).broadcast_to(
                                (128, N_JT, 64)))

                def qkv_unit(b, icl):
                    """QKV matmuls for one i-chunk + vn transpose chunk.
                    All of its DMAs ride qSP in production order."""
                    i0 = IC_W * (4 * b + icl)
                    if b == 0 and icl == 0:
                        xts = xt00  # pre-issued with the w3 loads
                    else:
                        xts = xtpool.tile([128, 8 * IC_W], F16, tag="xt",
                                          name=f"xt_{rep}_{b}_{icl}")
                        nc.sync.dma_start(
                            xts[:].rearrange("p (ct c) -> p ct c", ct=8),
                            xt_d.ap()[:, i0:i0 + IC_W]
                                .rearrange("(ct p) c -> p ct c", p=128))
                    for n, dst in enumerate((qt2b[b], kt2b[b], vt2b[b])):
                        ps = ps_mm.tile([128, IC_W], F32, tag="mm",
                                        name=f"psmm_{rep}_{b}_{icl}_{n}")
                        for ct in range(8):
                            nc.tensor.matmul(
                                ps[:],
                                w3_sb[:, 384 * ct + 128 * n:
                                      384 * ct + 128 * (n + 1)],
                                xts[:, IC_W * ct:IC_W * (ct + 1)],
                                start=(ct == 0), stop=(ct == 7))
                        nc.vector.tensor_scalar_add(
                            dst[:, IC_W * icl:IC_W * (icl + 1)], ps[:],
                            bias_sb[:, n:n + 1])
                    # v key-major via XBAR DMA transpose: [128keys,64dims]
                    # blocks land at stride 128 (ones cols pre-filled above)
                    for h in range(HPC):
                        nc.sync.dma_start(
                            vn_sb[(b, h)][:, 512 * icl:512 * (icl + 1)]
                                .rearrange("p (jt d) -> p jt d", d=128)
                                [:, :, 0:64],
                            vt2b[b][64 * h:64 * (h + 1),
                                    IC_W * icl:IC_W * (icl + 1)],
                            transpose=True)

                def attn_unit(b, icl):
                    """Attention + out projection for one i-chunk."""
                    i0 = IC_W * icl
                    njt = 4 * icl + 4 if causal else N_JT
                    pso = [ps_o.tile([128, IC_W], F32, tag="o",
                                     name=f"pso_{rep}_{b}_{icl}_{h_}")
                           for h_ in range(HPC)]

                    for jt in range(njt):
                        j0 = JT_W * jt
                        r_ = jt - 4 * icl
                        qs = JT_W * r_ if (causal and r_ > 0) else 0
                        for h in range(HPC):
                            h0 = 64 * h
                            pss = ps_s.tile([128, IC_W], F32, tag="s")
                            nc.tensor.matmul(
                                pss[:, qs:IC_W],
                                kt2b[b][h0:h0 + 64, j0:j0 + JT_W],
                                qt2b[b][h0:h0 + 64, i0 + qs:i0 + IC_W],
                                start=True, stop=True,
                                tile_position=(h0, 0))
                            pt = ptpool.tile([128, IC_W], F16, tag="pt")
                            nc.scalar.activation(
                                pt[:, qs:IC_W], pss[:, qs:IC_W],
                                mybir.ActivationFunctionType.Exp)
                            if causal and r_ >= 0:
                                # Pool: pt is SBUF; keeps the DVE queue
                                # free for psum-reading work
                                nc.gpsimd.tensor_mul(
                                    pt[:, qs:qs + JT_W],
                                    pt[:, qs:qs + JT_W], msk_sb[:])
                            nc.tensor.matmul(
                                pso[h][:, qs:IC_W],
                                vn_sb[(b, h)][:, 128 * jt:128 * (jt + 1)],
                                pt[:, qs:IC_W],
                                start=(jt == 0), stop=(jt == njt - 1),
                                tile_position=(0, 0),
                                skip_group_check=True)
                    # normalize -> ot [128, 512] (h0 rows 0:64, h1 64:128)
                    ot = otpool.tile([128, IC_W], F16, tag="ot",
                                     name=f"ot_{rep}_{b}_{icl}")
                    for h in range(HPC):
                        rcb = rcbpool.tile([64, IC_W], F32, tag="rcb",
                                           name=f"rcb_{rep}_{b}_{icl}_{h}")
                        nc.vector.reciprocal(rcb[:], pso[h][64:128, :])
                        nc.vector.tensor_mul(
                            ot[64 * h:64 * (h + 1), :], pso[h][0:64, :],
                            rcb[:])
                    # projection: partial[i0b:i0b+512, :] = ot^T @ wo
                    # psum->SBUF fp16 copies split DVE/ACT (Pool can't read
                    # PSUM); the out DMA goes through SWDGE on Pool so it
                    # never blocks qSP prefetches or the ACT exp stream
                    osb = opool.tile([128, 4 * C], F16, tag="osb",
                                     name=f"osb_{rep}_{b}_{icl}")
                    for it in range(4):
                        for oc in range(2):
                            psp = ps_p.tile([128, IC_W], F32, tag="p")
                            nc.tensor.matmul(
                                psp[:],
                                ot[:, 128 * it:128 * (it + 1)],
                                wo_sb[:, IC_W * oc:IC_W * (oc + 1)],
                                start=True, stop=True)
                            dst = osb[:, C * it + IC_W * oc:
                                      C * it + IC_W * (oc + 1)]
                            if (it * 2 + oc) % 4 == 3:
                                nc.scalar.activation(
                                    dst, psp[:],
                                    mybir.ActivationFunctionType.Copy)
                            else:
                                nc.vector.tensor_copy(dst, psp[:])
                    r0 = T * b + i0
                    # last chunk: split across the (by then idle) HWDGE
                    # queues to shorten the kernel tail
                    last = (b == B - 1 and icl == 3)
                    nsplit = 2 if last else 1
                    for oh in range(nsplit):
                        iw = 4 // nsplit
                        eng = (nc.sync, nc.scalar)[oh] if last else nc.gpsimd
                        eng.dma_start(
                            out_d.ap()[r0 + 128 * iw * oh:
                                       r0 + 128 * iw * (oh + 1), :]
                                .rearrange("(it p) c -> p it c", p=128),
                            osb[:, C * iw * oh:C * iw * (oh + 1)]
                                .rearrange("p (it c) -> p it c", it=iw))

                # emission order: b0 qkv, then b1 qkv interleaved with the
                # attention units of both batches (attention is ACT-paced, so
                # feeding PE qkv/oproj work between chains keeps it busy)
                for icl in range(4):
                    qkv_unit(0, icl)
                plan = [("q", 1, 0), ("a", 0, 0), ("q", 1, 1), ("a", 0, 1),
                        ("q", 1, 2), ("a", 1, 0), ("q", 1, 3), ("a", 0, 2),
                        ("a", 1, 1), ("a", 0, 3), ("a", 1, 2), ("a", 1, 3)]
                for kind, b, icl in plan:
                    if kind == "q":
                        qkv_unit(b, icl)
                    else:
                        attn_unit(b, icl)
    nc.compile()
    return nc


_PROGS = {}


def _get_prog(causal: bool, reps: int = 1):
    key = (causal, reps)
    if key not in _PROGS:
        _PROGS[key] = _build(causal, reps)
    return _PROGS[key]


def _prep_inputs(x, Wqkv, bqkv, Wo):
    """Per-core input maps (host-side sharding)."""
    x = np.asarray(x, dtype=np.float32)
    Wqkv = np.asarray(Wqkv, dtype=np.float32)
    bqkv = np.asarray(bqkv, dtype=np.float32)
    Wo = np.asarray(Wo, dtype=np.float32)

    xt = np.ascontiguousarray(x.reshape(R, C).T).astype(np.float16)  # [C, R]

    # causal mask for the residual 128x128 triangle of a diagonal j-tile,
    # plus a [128, 64] ones block, packed as one [128, 192] constant
    jl = np.arange(JT_W)[:, None]
    il = np.arange(JT_W)[None, :]
    msk = (jl <= il).astype(np.float16)
    mskones = np.concatenate(
        [msk, np.ones((128, 64), dtype=np.float16)], axis=1)

    in_maps = []
    scale = 1.0 / np.sqrt(np.float32(HS))
    for m in range(N_CORES):
        h0, h1 = HPC * m, HPC * m + 1
        cols = {}
        for name, off, sc in (("q", 0, scale), ("k", HS, 1.0), ("v", 2 * HS, 1.0)):
            blk = [Wqkv[:, 192 * h + off:192 * h + off + HS] * sc
                   for h in (h0, h1)]
            bb = [bqkv[192 * h + off:192 * h + off + HS] * sc for h in (h0, h1)]
            cols[name] = (np.concatenate(blk, axis=1),
                          np.concatenate(bb))
        w3 = np.concatenate([cols["q"][0], cols["k"][0], cols["v"][0]], axis=1)
        b3 = np.stack([cols["q"][1], cols["k"][1], cols["v"][1]], axis=1)
        wo = Wo[128 * m:128 * (m + 1), :]
        in_maps.append({
            "xt": xt,
            "w3": np.ascontiguousarray(w3).astype(np.float16),
            "b3": np.ascontiguousarray(b3.astype(np.float32)),  # [128, 3]
            "wo": np.ascontiguousarray(wo).astype(np.float16),
            "mskones": mskones,
        })
    return in_maps


class _Runner:
    """Cached shard_map runner for the SPMD NEFF (avoids re-jit per call)."""

    def __init__(self, nc, donate=True):
        import jax
        from jax.sharding import Mesh, PartitionSpec
        from jax.experimental.shard_map import shard_map
        from concourse import bass2jax

        bass2jax.install_neuronx_cc_hook()

        part_name = (nc.partition_id_tensor.name
                     if nc.partition_id_tensor else None)
        in_names, out_names, out_avals, zero_outs = [], [], [], []
        for alloc in nc.m.functions[0].allocations:
            if not isinstance(alloc, mybir.MemoryLocationSet):
                continue
            name = alloc.memorylocations[0].name
            if alloc.kind == "ExternalInput":
                if name != part_name:
                    in_names.append(name)
            elif alloc.kind == "ExternalOutput":
                out_names.append(name)
                shape = tuple(alloc.tensor_shape)
                dtype = mybir.dt.np(alloc.dtype)
                out_avals.append(jax.core.ShapedArray(shape, dtype))
                zero_outs.append(np.zeros(shape, dtype))
        self.in_names, self.out_names = in_names, out_names
        self.zero_outs = zero_outs
        n_params, n_outs = len(in_names), len(out_names)
        all_in_names = tuple(in_names) + tuple(out_names)
        if part_name is not None:
            all_in_names = all_in_names + (part_name,)

        def _exec(args, outs):
            operands = list(args) + list(outs)
            if part_name is not None:
                operands.append(bass2jax.partition_id_tensor())
            return bass2jax._bass_exec_p.bind(
                *operands,
                out_avals=tuple(out_avals),
                in_names=all_in_names,
                out_names=tuple(out_names),
                lowering_input_output_aliases=(),
                sim_require_finite=True,
                sim_require_nnan=True,
                nc=nc)

        def _body(*args):
            ins, outs = args[:n_params], list(args[n_params:])
            return tuple(_exec(ins, outs))

        devices = jax.devices()[:N_CORES]
        mesh = Mesh(np.asarray(devices), ("core",))
        donate_kw = {}
        if donate:
            donate_kw["donate_argnums"] = tuple(
                range(n_params, n_params + n_outs))
        self._fn = jax.jit(
            shard_map(_body, mesh=mesh,
                      in_specs=(PartitionSpec("core"),) * (n_params + n_outs),
                      out_specs=(PartitionSpec("core"),) * n_outs,
                      check_rep=False),
            keep_unused=True, **donate_kw)

    def __call__(self, in_maps):
        concat_in = [
            np.concatenate([in_maps[c][k] for c in range(N_CORES)], axis=0)
            for k in self.in_names]
        concat_zero = [
            np.zeros((N_CORES * z.shape[0], *z.shape[1:]), z.dtype)
            for z in self.zero_outs]
        out = self._fn(*concat_in, *concat_zero)
        return [
            {k: np.asarray(out[i]).reshape(N_CORES, *self.zero_outs[i].shape)[c]
             for i, k in enumerate(self.out_names)}
            for c in range(N_CORES)]


_RUNNERS = {}


def _get_runner(causal: bool, reps: int = 1, donate: bool = True):
    key = (causal, reps, donate)
    if key not in _RUNNERS:
        _RUNNERS[key] = _Runner(_get_prog(causal, reps), donate=donate)
    return _RUNNERS[key]


def kernel(x, Wqkv, bqkv, Wo, bo, mask):
    causal = bool(np.asarray(mask).item()) if not isinstance(mask, (int, bool)) else bool(mask)
    runner = _get_runner(causal)
    in_maps = _prep_inputs(x, Wqkv, bqkv, Wo)
    results = runner(in_maps)
    acc = np.zeros((R, C), dtype=np.float32)
    for m in range(N_CORES):
        acc += results[m]["part"].astype(np.float32)
    acc += np.asarray(bo, dtype=np.float32)[None, :]
    return acc.reshape(B, T, C)


# revision 46
# speedup vs baseline: 1.1674x; 1.1674x over previous
"""Multi-head self-attention (B=2, T=2048, C=1024, H=16) on 8 TRN2 NeuronCores.

Sharding: tensor-parallel over heads. Core m owns heads (2m, 2m+1):
  - qkv^T = (Wqkv_shard^T) @ x^T for its 2 heads (contraction-major layouts;
    host pre-transposes x), fp16 operands, fp32 PSUM accumulation
  - causal attention, flash-style with blockwise exp (no max-subtraction:
    scores are O(1) here); the AV stationary packs [v-dims | 64 ones-columns]
    so the softmax denominator comes out broadcast across 64 PSUM partitions
  - causal query-slicing: diagonal-band blocks only stream queries >= key
    block start; the residual 128x128 triangle is masked on DVE
  - v^T -> v (key-major) via XBAR DMA-transpose (16-bit), no PE transposes
  - partial output projection partial_m = values_m @ Wo[rows of heads m],
    DMA'd straight from PSUM (fp32)
Host sums the 8 partials and adds bias bo.
"""

import numpy as np

import concourse.bass as bass
import concourse.bacc as bacc
import concourse.mybir as mybir
import concourse.tile as tile
from concourse.bass_utils import run_bass_kernel_spmd

B, T, C = 2, 2048, 1024
H, HS = 16, 64
N_CORES = 8
HPC = H // N_CORES            # heads per core = 2
R = B * T                      # 4096 rows total
IC_W = 512                     # i-chunk width (query cols per block)
JT_W = 128                     # j-tile width (key rows per block)
N_IC = T // IC_W               # 4 i-chunks per batch
N_JT = T // JT_W               # 16 j-tiles per batch
F32 = mybir.dt.float32
F32R = mybir.dt.float32r
F16 = mybir.dt.float16


def _build(causal: bool, reps: int = 1):
    nc = bacc.Bacc("TRN2", target_bir_lowering=False, debug=False,
                   num_devices=N_CORES)

    # xt pre-tiled host-side: [128p, 8ic x 8ct x 512] so every i-chunk load
    # is one fully-contiguous-per-partition DMA (8KB runs, 128 descriptors)
    xt_d = nc.dram_tensor("xt", [128, 8 * 8 * IC_W], F16, kind="ExternalInput")
    w3_d = nc.dram_tensor("w3", [128, 8 * 384], F16, kind="ExternalInput")
    b3_d = nc.dram_tensor("b3", [128, 3], F32, kind="ExternalInput")
    wo_d = nc.dram_tensor("wo", [128, C], F16, kind="ExternalInput")
    # cols 0:128 = causal 128x128 tril mask; cols 128:192 = ones
    mo_d = nc.dram_tensor("mskones", [128, JT_W + 64], F16,
                          kind="ExternalInput")
    # partial output pre-tiled: [8ic, 128p, 4it x 1024c]; host inverts
    out_d = nc.dram_tensor("part", [8, 128, 4 * C], F16, kind="ExternalOutput")

    with tile.TileContext(nc) as tc:
        with (
            tc.tile_pool(name="const", bufs=1) as cpool,
            tc.tile_pool(name="qkv", bufs=1) as qkvpool,
            tc.tile_pool(name="xt", bufs=6) as xtpool,
            tc.tile_pool(name="pt", bufs=12) as ptpool,
            tc.tile_pool(name="rcb", bufs=2) as rcbpool,
            tc.tile_pool(name="ot", bufs=2) as otpool,
            tc.tile_pool(name="osb", bufs=2) as opool,
            tc.tile_pool(name="ps_mm", bufs=2, space="PSUM") as ps_mm,
            tc.tile_pool(name="ps_s", bufs=3, space="PSUM") as ps_s,
            tc.tile_pool(name="ps_o", bufs=2, space="PSUM") as ps_o,
            tc.tile_pool(name="ps_p", bufs=1, space="PSUM") as ps_p,
        ):
            for rep in range(reps):
                # ---- constants: qSP gets w3/b3 (+x tiles below); qACT the rest
                # w3 and the first x chunk split + interleaved so the first
                # QKV matmuls start as early as possible
                w3_sb = cpool.tile([128, 8 * 384], F16, tag="w3")
                xt00 = xtpool.tile([128, 8 * IC_W], F16, tag="xt",
                                   name=f"xt_{rep}_0_0")
                for wh in range(2):
                    # w3 rides qSP while the first x chunk rides qACT, so
                    # the two startup streams land in parallel
                    nc.sync.dma_start(
                        w3_sb[:, 1536 * wh:1536 * (wh + 1)],
                        w3_d.ap()[:, 1536 * wh:1536 * (wh + 1)])
                    nc.scalar.dma_start(
                        xt00[:, 2048 * wh:2048 * (wh + 1)],
                        xt_d.ap()[:, 2048 * wh:2048 * (wh + 1)])
                bias_sb = cpool.tile([128, 3], F32, tag="b3")
                nc.sync.dma_start(bias_sb[:], b3_d.ap()[:])
                wo_sb = cpool.tile([128, C], F16, tag="wo")
                nc.scalar.dma_start(wo_sb[:], wo_d.ap()[:])
                mo_sb = cpool.tile([128, JT_W + 64], F16, tag="mskones")
                nc.scalar.dma_start(mo_sb[:], mo_d.ap()[:])
                msk_sb = mo_sb[:, 0:JT_W]
                ones_sb = mo_sb[:, JT_W:JT_W + 64]

                # persistent qkv^T (transposed layouts, heads packed 2-up)
                qt2b = [qkvpool.tile([128, T], F16, tag=f"qt2_{b_}",
                                     name=f"qt2_{rep}_{b_}")
                        for b_ in range(B)]
                kt2b = [qkvpool.tile([128, T], F16, tag=f"kt2_{b_}",
                                     name=f"kt2_{rep}_{b_}")
                        for b_ in range(B)]
                vt2b = [qkvpool.tile([128, T], F16, tag=f"vt2_{b_}",
                                     name=f"vt2_{rep}_{b_}")
                        for b_ in range(B)]
                # v in key-major layout + ones cols: per (b,h) [128, 16*128]
                # each 128-col block: [0:64) = v dims, [64:128) = ones
                vn_sb = {(b, h): qkvpool.tile([128, N_JT * 128], F16,
                                              tag=f"vn_{b}_{h}",
                                              name=f"vn_{rep}_{b}_{h}")
                         for b in range(B) for h in range(HPC)}

                # ones cols fill [64:128) of every vn 128-col block; on Pool
                # (SBUF->SBUF) so no compute queue ever stalls the first AVs
                for b in range(B):
                    for h in range(HPC):
                        nc.gpsimd.tensor_copy(
                            vn_sb[(b, h)][:]
                                .rearrange("p (jt d) -> p jt d", d=128)
                                [:, :, 64:128],
                            ones_sb.unsqueeze(1).broadcast_to(
                                (128, N_JT, 64)))

                def qkv_unit(b, icl):
                    """QKV matmuls for one i-chunk + vn transpose chunk.
                    All of its DMAs ride qSP in production order."""
                    i0 = IC_W * (4 * b + icl)
                    if b == 0 and icl == 0:
                        xts = xt00  # pre-issued with the w3 loads
                    else:
                        xts = xtpool.tile([128, 8 * IC_W], F16, tag="xt",
                                          name=f"xt_{rep}_{b}_{icl}")
                        ic = 4 * b + icl
                        nc.sync.dma_start(
                            xts[:],
                            xt_d.ap()[:, 4096 * ic:4096 * (ic + 1)])
                    for n, dst in enumerate((qt2b[b], kt2b[b], vt2b[b])):
                        ps = ps_mm.tile([128, IC_W], F32, tag="mm",
                                        name=f"psmm_{rep}_{b}_{icl}_{n}")
                        for ct in range(8):
                            nc.tensor.matmul(
                                ps[:],
                                w3_sb[:, 384 * ct + 128 * n:
                                      384 * ct + 128 * (n + 1)],
                                xts[:, IC_W * ct:IC_W * (ct + 1)],
                                start=(ct == 0), stop=(ct == 7))
                        nc.vector.tensor_scalar_add(
                            dst[:, IC_W * icl:IC_W * (icl + 1)], ps[:],
                            bias_sb[:, n:n + 1])
                    # v key-major via XBAR DMA transpose: [128keys,64dims]
                    # blocks land at stride 128 (ones cols pre-filled above)
                    for h in range(HPC):
                        nc.sync.dma_start(
                            vn_sb[(b, h)][:, 512 * icl:512 * (icl + 1)]
                                .rearrange("p (jt d) -> p jt d", d=128)
                                [:, :, 0:64],
                            vt2b[b][64 * h:64 * (h + 1),
                                    IC_W * icl:IC_W * (icl + 1)],
                            transpose=True)

                def attn_unit(b, icl):
                    """Attention + out projection for one i-chunk."""
                    i0 = IC_W * icl
                    njt = 4 * icl + 4 if causal else N_JT
                    pso = [ps_o.tile([128, IC_W], F32, tag="o",
                                     name=f"pso_{rep}_{b}_{icl}_{h_}")
                           for h_ in range(HPC)]

                    for jt in range(njt):
                        j0 = JT_W * jt
                        r_ = jt - 4 * icl
                        qs = JT_W * r_ if (causal and r_ > 0) else 0
                        for h in range(HPC):
                            h0 = 64 * h
                            pss = ps_s.tile([128, IC_W], F32, tag="s")
                            nc.tensor.matmul(
                                pss[:, qs:IC_W],
                                kt2b[b][h0:h0 + 64, j0:j0 + JT_W],
                                qt2b[b][h0:h0 + 64, i0 + qs:i0 + IC_W],
                                start=True, stop=True)
                            pt = ptpool.tile([128, IC_W], F16, tag="pt")
                            nc.scalar.activation(
                                pt[:, qs:IC_W], pss[:, qs:IC_W],
                                mybir.ActivationFunctionType.Exp)
                            if causal and r_ >= 0:
                                # Pool: pt is SBUF; keeps the DVE queue
                                # free for psum-reading work
                                nc.gpsimd.tensor_mul(
                                    pt[:, qs:qs + JT_W],
                                    pt[:, qs:qs + JT_W], msk_sb[:])
                            nc.tensor.matmul(
                                pso[h][:, qs:IC_W],
                                vn_sb[(b, h)][:, 128 * jt:128 * (jt + 1)],
                                pt[:, qs:IC_W],
                                start=(jt == 0), stop=(jt == njt - 1),
                                tile_position=(0, 0),
                                skip_group_check=True)
                    # normalize -> ot [128, 512] (h0 rows 0:64, h1 64:128)
                    ot = otpool.tile([128, IC_W], F16, tag="ot",
                                     name=f"ot_{rep}_{b}_{icl}")
                    for h in range(HPC):
                        rcb = rcbpool.tile([64, IC_W], F32, tag="rcb",
                                           name=f"rcb_{rep}_{b}_{icl}_{h}")
                        nc.vector.reciprocal(rcb[:], pso[h][64:128, :])
                        nc.vector.tensor_mul(
                            ot[64 * h:64 * (h + 1), :], pso[h][0:64, :],
                            rcb[:])
                    # projection: partial[i0b:i0b+512, :] = ot^T @ wo
                    # psum->SBUF fp16 copies split DVE/ACT (Pool can't read
                    # PSUM); the out DMA goes through SWDGE on Pool so it
                    # never blocks qSP prefetches or the ACT exp stream
                    osb = opool.tile([128, 4 * C], F16, tag="osb",
                                     name=f"osb_{rep}_{b}_{icl}")
                    last = (b == B - 1 and icl == 3)
                    for it in range(4):
                        for oc in range(2):
                            # the final unit recycles the freed pso banks for
                            # 2-deep oproj pipelining (ps_p is 1 bank), and
                            # splits its copies DVE/ACT to shorten the tail
                            pool = ps_o if last else ps_p
                            psp = pool.tile([128, IC_W], F32,
                                            tag="o" if last else "p",
                                            name=f"psp_{rep}_{b}_{icl}_{it}_{oc}")
                            nc.tensor.matmul(
                                psp[:],
                                ot[:, 128 * it:128 * (it + 1)],
                                wo_sb[:, IC_W * oc:IC_W * (oc + 1)],
                                start=True, stop=True)
                            dst = osb[:, C * it + IC_W * oc:
                                      C * it + IC_W * (oc + 1)]
                            if last and (it * 2 + oc) % 2 == 1:
                                nc.scalar.activation(
                                    dst, psp[:],
                                    mybir.ActivationFunctionType.Copy)
                            else:
                                nc.vector.tensor_copy(dst, psp[:])
                    ic = 4 * b + icl
                    # last chunk: split across the (by then idle) HWDGE
                    # queues to shorten the kernel tail
                    nsplit = 2 if last else 1
                    for oh in range(nsplit):
                        w = 4 * C // nsplit
                        eng = (nc.sync, nc.scalar)[oh] if last else nc.gpsimd
                        eng.dma_start(
                            out_d.ap()[ic][:, w * oh:w * (oh + 1)],
                            osb[:, w * oh:w * (oh + 1)])

                # emission order: b0 qkv, then b1 qkv interleaved with the
                # attention units of both batches (attention is ACT-paced, so
                # feeding PE qkv/oproj work between chains keeps it busy)
                for icl in range(4):
                    qkv_unit(0, icl)
                plan = [("q", 1, 0), ("a", 0, 0), ("q", 1, 1), ("a", 0, 1),
                        ("q", 1, 2), ("a", 1, 0), ("q", 1, 3), ("a", 0, 2),
                        ("a", 1, 1), ("a", 0, 3), ("a", 1, 2), ("a", 1, 3)]
                for kind, b, icl in plan:
                    if kind == "q":
                        qkv_unit(b, icl)
                    else:
                        attn_unit(b, icl)
    nc.compile()
    return nc


_PROGS = {}


def _get_prog(causal: bool, reps: int = 1):
    key = (causal, reps)
    if key not in _PROGS:
        _PROGS[key] = _build(causal, reps)
    return _PROGS[key]


def _prep_inputs(x, Wqkv, bqkv, Wo):
    """Per-core input maps (host-side sharding)."""
    x = np.asarray(x, dtype=np.float32)
    Wqkv = np.asarray(Wqkv, dtype=np.float32)
    bqkv = np.asarray(bqkv, dtype=np.float32)
    Wo = np.asarray(Wo, dtype=np.float32)

    # x^T pre-tiled: xt2[p, ic, ct, col] = x^T[128*ct + p, 512*ic + col]
    # -> every i-chunk's load is contiguous per partition
    xt = np.ascontiguousarray(x.reshape(R, C).T).astype(np.float16)  # [C, R]
    xt2 = np.ascontiguousarray(
        xt.reshape(8, 128, 8, IC_W).transpose(1, 2, 0, 3)
    ).reshape(128, 8 * 8 * IC_W)

    # causal mask for the residual 128x128 triangle of a diagonal j-tile,
    # plus a [128, 64] ones block, packed as one [128, 192] constant
    jl = np.arange(JT_W)[:, None]
    il = np.arange(JT_W)[None, :]
    msk = (jl <= il).astype(np.float16)
    mskones = np.concatenate(
        [msk, np.ones((128, 64), dtype=np.float16)], axis=1)

    in_maps = []
    scale = 1.0 / np.sqrt(np.float32(HS))
    for m in range(N_CORES):
        h0, h1 = HPC * m, HPC * m + 1
        cols = {}
        for name, off, sc in (("q", 0, scale), ("k", HS, 1.0), ("v", 2 * HS, 1.0)):
            blk = [Wqkv[:, 192 * h + off:192 * h + off + HS] * sc
                   for h in (h0, h1)]
            bb = [bqkv[192 * h + off:192 * h + off + HS] * sc for h in (h0, h1)]
            cols[name] = (np.concatenate(blk, axis=1),
                          np.concatenate(bb))
        w3 = np.concatenate([cols["q"][0], cols["k"][0], cols["v"][0]], axis=1)
        # pre-tile like xt: w3t[p, ct*384 + col] = w3[128*ct + p, col]
        w3t = np.ascontiguousarray(
            w3.astype(np.float16).reshape(8, 128, 384).transpose(1, 0, 2)
        ).reshape(128, 8 * 384)
        b3 = np.stack([cols["q"][1], cols["k"][1], cols["v"][1]], axis=1)
        wo = Wo[128 * m:128 * (m + 1), :]
        in_maps.append({
            "xt": xt2,
            "w3": w3t,
            "b3": np.ascontiguousarray(b3.astype(np.float32)),  # [128, 3]
            "wo": np.ascontiguousarray(wo).astype(np.float16),
            "mskones": mskones,
        })
    return in_maps


class _Runner:
    """Cached shard_map runner for the SPMD NEFF (avoids re-jit per call)."""

    def __init__(self, nc, donate=True):
        import jax
        from jax.sharding import Mesh, PartitionSpec
        from jax.experimental.shard_map import shard_map
        from concourse import bass2jax

        bass2jax.install_neuronx_cc_hook()

        part_name = (nc.partition_id_tensor.name
                     if nc.partition_id_tensor else None)
        in_names, out_names, out_avals, zero_outs = [], [], [], []
        for alloc in nc.m.functions[0].allocations:
            if not isinstance(alloc, mybir.MemoryLocationSet):
                continue
            name = alloc.memorylocations[0].name
            if alloc.kind == "ExternalInput":
                if name != part_name:
                    in_names.append(name)
            elif alloc.kind == "ExternalOutput":
                out_names.append(name)
                shape = tuple(alloc.tensor_shape)
                dtype = mybir.dt.np(alloc.dtype)
                out_avals.append(jax.core.ShapedArray(shape, dtype))
                zero_outs.append(np.zeros(shape, dtype))
        self.in_names, self.out_names = in_names, out_names
        self.zero_outs = zero_outs
        n_params, n_outs = len(in_names), len(out_names)
        all_in_names = tuple(in_names) + tuple(out_names)
        if part_name is not None:
            all_in_names = all_in_names + (part_name,)

        def _exec(args, outs):
            operands = list(args) + list(outs)
            if part_name is not None:
                operands.append(bass2jax.partition_id_tensor())
            return bass2jax._bass_exec_p.bind(
                *operands,
                out_avals=tuple(out_avals),
                in_names=all_in_names,
                out_names=tuple(out_names),
                lowering_input_output_aliases=(),
                sim_require_finite=True,
                sim_require_nnan=True,
                nc=nc)

        def _body(*args):
            ins, outs = args[:n_params], list(args[n_params:])
            return tuple(_exec(ins, outs))

        devices = jax.devices()[:N_CORES]
        mesh = Mesh(np.asarray(devices), ("core",))
        donate_kw = {}
        if donate:
            donate_kw["donate_argnums"] = tuple(
                range(n_params, n_params + n_outs))
        self._fn = jax.jit(
            shard_map(_body, mesh=mesh,
                      in_specs=(PartitionSpec("core"),) * (n_params + n_outs),
                      out_specs=(PartitionSpec("core"),) * n_outs,
                      check_rep=False),
            keep_unused=True, **donate_kw)

    def __call__(self, in_maps):
        concat_in = [
            np.concatenate([in_maps[c][k] for c in range(N_CORES)], axis=0)
            for k in self.in_names]
        concat_zero = [
            np.zeros((N_CORES * z.shape[0], *z.shape[1:]), z.dtype)
            for z in self.zero_outs]
        out = self._fn(*concat_in, *concat_zero)
        return [
            {k: np.asarray(out[i]).reshape(N_CORES, *self.zero_outs[i].shape)[c]
             for i, k in enumerate(self.out_names)}
            for c in range(N_CORES)]


_RUNNERS = {}


def _get_runner(causal: bool, reps: int = 1, donate: bool = True):
    key = (causal, reps, donate)
    if key not in _RUNNERS:
        _RUNNERS[key] = _Runner(_get_prog(causal, reps), donate=donate)
    return _RUNNERS[key]


def kernel(x, Wqkv, bqkv, Wo, bo, mask):
    causal = bool(np.asarray(mask).item()) if not isinstance(mask, (int, bool)) else bool(mask)
    runner = _get_runner(causal)
    in_maps = _prep_inputs(x, Wqkv, bqkv, Wo)
    results = runner(in_maps)
    acc = np.zeros((R, C), dtype=np.float32)
    for m in range(N_CORES):
        # invert the [8ic, 128p, 4it*1024c] output tiling
        part = results[m]["part"].reshape(8, 128, 4, C) \
            .transpose(0, 2, 1, 3).reshape(R, C)
        acc += part.astype(np.float32)
    acc += np.asarray(bo, dtype=np.float32)[None, :]
    return acc.reshape(B, T, C)


# revision 47
# speedup vs baseline: 1.4041x; 1.2028x over previous
"""Multi-head self-attention (B=2, T=2048, C=1024, H=16) on 8 TRN2 NeuronCores.

Sharding: tensor-parallel over heads. Core m owns heads (2m, 2m+1):
  - qkv^T = (Wqkv_shard^T) @ x^T for its 2 heads (contraction-major layouts;
    host pre-transposes + pre-tiles x so every load is one contiguous
    per-partition DMA), fp16 operands, fp32 PSUM accumulation
  - causal attention, flash-style with blockwise exp (no max-subtraction:
    scores are O(1) here); the AV stationary packs [v-dims | 64 ones-columns]
    so the softmax denominator comes out broadcast across 64 PSUM partitions
  - causal query-slicing: diagonal-band blocks only stream queries >= key
    block start; the residual 128x128 triangle is masked on Pool
  - v^T -> v (key-major) via XBAR DMA-transpose (16-bit), no PE transposes
  - partial output projection partial_m = values_m @ Wo[rows of heads m];
    psum->SBUF fp16 copies on DVE, stores via SWDGE so neither HWDGE input
    queue nor the ACT exp stream ever blocks
  - the two batches' attention i-chunks are emitted interleaved with the
    second batch's QKV so the PE always has off-critical-path work while
    the ACT engine streams exps
Host sums the 8 fp16 partials (inverting the output tiling) and adds bo.
"""

import numpy as np

import concourse.bass as bass
import concourse.bacc as bacc
import concourse.mybir as mybir
import concourse.tile as tile
from concourse.bass_utils import run_bass_kernel_spmd

B, T, C = 2, 2048, 1024
H, HS = 16, 64
N_CORES = 8
HPC = H // N_CORES            # heads per core = 2
R = B * T                      # 4096 rows total
IC_W = 512                     # i-chunk width (query cols per block)
JT_W = 128                     # j-tile width (key rows per block)
N_IC = T // IC_W               # 4 i-chunks per batch
N_JT = T // JT_W               # 16 j-tiles per batch
F32 = mybir.dt.float32
F32R = mybir.dt.float32r
F16 = mybir.dt.float16


def _build(causal: bool, reps: int = 1):
    nc = bacc.Bacc("TRN2", target_bir_lowering=False, debug=False,
                   num_devices=N_CORES)

    # xt pre-tiled host-side: [128p, 8ic x 8ct x 512] so every i-chunk load
    # is one fully-contiguous-per-partition DMA (8KB runs, 128 descriptors)
    xt_d = nc.dram_tensor("xt", [128, 8 * 8 * IC_W], F16, kind="ExternalInput")
    w3_d = nc.dram_tensor("w3", [128, 8 * 384], F16, kind="ExternalInput")
    b3_d = nc.dram_tensor("b3", [128, 3], F32, kind="ExternalInput")
    wo_d = nc.dram_tensor("wo", [128, C], F16, kind="ExternalInput")
    # cols 0:128 = causal 128x128 tril mask; cols 128:192 = ones
    mo_d = nc.dram_tensor("mskones", [128, JT_W + 64], F16,
                          kind="ExternalInput")
    # partial output pre-tiled: [8ic, 128p, 4it x 1024c]; host inverts
    out_d = nc.dram_tensor("part", [8, 128, 4 * C], F16, kind="ExternalOutput")

    with tile.TileContext(nc) as tc:
        with (
            tc.tile_pool(name="const", bufs=1) as cpool,
            tc.tile_pool(name="qkv", bufs=1) as qkvpool,
            tc.tile_pool(name="xt", bufs=6) as xtpool,
            tc.tile_pool(name="pt", bufs=12) as ptpool,
            tc.tile_pool(name="rcb", bufs=2) as rcbpool,
            tc.tile_pool(name="ot", bufs=2) as otpool,
            tc.tile_pool(name="osb", bufs=2) as opool,
            tc.tile_pool(name="ps_mm", bufs=2, space="PSUM") as ps_mm,
            tc.tile_pool(name="ps_s", bufs=3, space="PSUM") as ps_s,
            tc.tile_pool(name="ps_o", bufs=2, space="PSUM") as ps_o,
            tc.tile_pool(name="ps_p", bufs=1, space="PSUM") as ps_p,
        ):
            for rep in range(reps):
                # ---- constants: qSP gets w3/b3 (+x tiles below); qACT the rest
                # w3 and the first x chunk split + interleaved so the first
                # QKV matmuls start as early as possible
                w3_sb = cpool.tile([128, 8 * 384], F16, tag="w3")
                xt00 = xtpool.tile([128, 8 * IC_W], F16, tag="xt",
                                   name=f"xt_{rep}_0_0")
                for wh in range(2):
                    # w3 rides qSP while the first x chunk rides qACT, so
                    # the two startup streams land in parallel
                    nc.sync.dma_start(
                        w3_sb[:, 1536 * wh:1536 * (wh + 1)],
                        w3_d.ap()[:, 1536 * wh:1536 * (wh + 1)])
                    nc.scalar.dma_start(
                        xt00[:, 2048 * wh:2048 * (wh + 1)],
                        xt_d.ap()[:, 2048 * wh:2048 * (wh + 1)])
                bias_sb = cpool.tile([128, 3], F32, tag="b3")
                nc.sync.dma_start(bias_sb[:], b3_d.ap()[:])
                wo_sb = cpool.tile([128, C], F16, tag="wo")
                nc.scalar.dma_start(wo_sb[:], wo_d.ap()[:])
                mo_sb = cpool.tile([128, JT_W + 64], F16, tag="mskones")
                nc.scalar.dma_start(mo_sb[:], mo_d.ap()[:])
                msk_sb = mo_sb[:, 0:JT_W]
                ones_sb = mo_sb[:, JT_W:JT_W + 64]

                # persistent qkv^T (transposed layouts, heads packed 2-up)
                qt2b = [qkvpool.tile([128, T], F16, tag=f"qt2_{b_}",
                                     name=f"qt2_{rep}_{b_}")
                        for b_ in range(B)]
                kt2b = [qkvpool.tile([128, T], F16, tag=f"kt2_{b_}",
                                     name=f"kt2_{rep}_{b_}")
                        for b_ in range(B)]
                vt2b = [qkvpool.tile([128, T], F16, tag=f"vt2_{b_}",
                                     name=f"vt2_{rep}_{b_}")
                        for b_ in range(B)]
                # v in key-major layout + ones cols: per (b,h) [128, 16*128]
                # each 128-col block: [0:64) = v dims, [64:128) = ones
                vn_sb = {(b, h): qkvpool.tile([128, N_JT * 128], F16,
                                              tag=f"vn_{b}_{h}",
                                              name=f"vn_{rep}_{b}_{h}")
                         for b in range(B) for h in range(HPC)}

                # ones cols fill [64:128) of every vn 128-col block; on Pool
                # (SBUF->SBUF) so no compute queue ever stalls the first AVs
                for b in range(B):
                    for h in range(HPC):
                        nc.gpsimd.tensor_copy(
                            vn_sb[(b, h)][:]
                                .rearrange("p (jt d) -> p jt d", d=128)
                                [:, :, 64:128],
                            ones_sb.unsqueeze(1).broadcast_to(
                                (128, N_JT, 64)))

                def qkv_unit(b, icl):
                    """QKV matmuls for one i-chunk + vn transpose chunk.
                    All of its DMAs ride qSP in production order."""
                    i0 = IC_W * (4 * b + icl)
                    if b == 0 and icl == 0:
                        xts = xt00  # pre-issued with the w3 loads
                    else:
                        xts = xtpool.tile([128, 8 * IC_W], F16, tag="xt",
                                          name=f"xt_{rep}_{b}_{icl}")
                        ic = 4 * b + icl
                        nc.sync.dma_start(
                            xts[:],
                            xt_d.ap()[:, 4096 * ic:4096 * (ic + 1)])
                    for n, dst in enumerate((qt2b[b], kt2b[b], vt2b[b])):
                        ps = ps_mm.tile([128, IC_W], F32, tag="mm",
                                        name=f"psmm_{rep}_{b}_{icl}_{n}")
                        for ct in range(8):
                            nc.tensor.matmul(
                                ps[:],
                                w3_sb[:, 384 * ct + 128 * n:
                                      384 * ct + 128 * (n + 1)],
                                xts[:, IC_W * ct:IC_W * (ct + 1)],
                                start=(ct == 0), stop=(ct == 7))
                        nc.vector.tensor_scalar_add(
                            dst[:, IC_W * icl:IC_W * (icl + 1)], ps[:],
                            bias_sb[:, n:n + 1])
                    # v key-major via XBAR DMA transpose: [128keys,64dims]
                    # blocks land at stride 128 (ones cols pre-filled above)
                    for h in range(HPC):
                        nc.sync.dma_start(
                            vn_sb[(b, h)][:, 512 * icl:512 * (icl + 1)]
                                .rearrange("p (jt d) -> p jt d", d=128)
                                [:, :, 0:64],
                            vt2b[b][64 * h:64 * (h + 1),
                                    IC_W * icl:IC_W * (icl + 1)],
                            transpose=True)

                def attn_unit(b, icl):
                    """Attention + out projection for one i-chunk."""
                    i0 = IC_W * icl
                    njt = 4 * icl + 4 if causal else N_JT
                    pso = [ps_o.tile([128, IC_W], F32, tag="o",
                                     name=f"pso_{rep}_{b}_{icl}_{h_}")
                           for h_ in range(HPC)]

                    for jt in range(njt):
                        j0 = JT_W * jt
                        r_ = jt - 4 * icl
                        qs = JT_W * r_ if (causal and r_ > 0) else 0
                        for h in range(HPC):
                            h0 = 64 * h
                            pss = ps_s.tile([128, IC_W], F32, tag="s")
                            nc.tensor.matmul(
                                pss[:, qs:IC_W],
                                kt2b[b][h0:h0 + 64, j0:j0 + JT_W],
                                qt2b[b][h0:h0 + 64, i0 + qs:i0 + IC_W],
                                start=True, stop=True)
                            pt = ptpool.tile([128, IC_W], F16, tag="pt")
                            nc.scalar.activation(
                                pt[:, qs:IC_W], pss[:, qs:IC_W],
                                mybir.ActivationFunctionType.Exp)
                            if causal and r_ >= 0:
                                # Pool: pt is SBUF; keeps the DVE queue
                                # free for psum-reading work
                                nc.gpsimd.tensor_mul(
                                    pt[:, qs:qs + JT_W],
                                    pt[:, qs:qs + JT_W], msk_sb[:])
                            nc.tensor.matmul(
                                pso[h][:, qs:IC_W],
                                vn_sb[(b, h)][:, 128 * jt:128 * (jt + 1)],
                                pt[:, qs:IC_W],
                                start=(jt == 0), stop=(jt == njt - 1),
                                tile_position=(0, 0),
                                skip_group_check=True)
                    # normalize -> ot [128, 512] (h0 rows 0:64, h1 64:128)
                    ot = otpool.tile([128, IC_W], F16, tag="ot",
                                     name=f"ot_{rep}_{b}_{icl}")
                    for h in range(HPC):
                        rcb = rcbpool.tile([64, IC_W], F32, tag="rcb",
                                           name=f"rcb_{rep}_{b}_{icl}_{h}")
                        nc.vector.reciprocal(rcb[:], pso[h][64:128, :])
                        nc.vector.tensor_mul(
                            ot[64 * h:64 * (h + 1), :], pso[h][0:64, :],
                            rcb[:])
                    # projection: partial[i0b:i0b+512, :] = ot^T @ wo
                    # psum->SBUF fp16 copies split DVE/ACT (Pool can't read
                    # PSUM); the out DMA goes through SWDGE on Pool so it
                    # never blocks qSP prefetches or the ACT exp stream
                    osb = opool.tile([128, 4 * C], F16, tag="osb",
                                     name=f"osb_{rep}_{b}_{icl}")
                    last = (b == B - 1 and icl == 3)
                    for it in range(4):
                        for oc in range(2):
                            # the final unit recycles the freed pso banks for
                            # 2-deep oproj pipelining (ps_p is 1 bank), and
                            # splits its copies DVE/ACT to shorten the tail
                            pool = ps_o if last else ps_p
                            psp = pool.tile([128, IC_W], F32,
                                            tag="o" if last else "p",
                                            name=f"psp_{rep}_{b}_{icl}_{it}_{oc}")
                            nc.tensor.matmul(
                                psp[:],
                                ot[:, 128 * it:128 * (it + 1)],
                                wo_sb[:, IC_W * oc:IC_W * (oc + 1)],
                                start=True, stop=True)
                            dst = osb[:, C * it + IC_W * oc:
                                      C * it + IC_W * (oc + 1)]
                            if last and (it * 2 + oc) % 2 == 1:
                                nc.scalar.activation(
                                    dst, psp[:],
                                    mybir.ActivationFunctionType.Copy)
                            else:
                                nc.vector.tensor_copy(dst, psp[:])
                    ic = 4 * b + icl
                    # last chunk: split across the (by then idle) HWDGE
                    # queues to shorten the kernel tail
                    nsplit = 2 if last else 1
                    for oh in range(nsplit):
                        w = 4 * C // nsplit
                        eng = (nc.sync, nc.scalar)[oh] if last else nc.gpsimd
                        eng.dma_start(
                            out_d.ap()[ic][:, w * oh:w * (oh + 1)],
                            osb[:, w * oh:w * (oh + 1)])

                # emission order: b0 qkv, then b1 qkv interleaved with the
                # attention units of both batches (attention is ACT-paced, so
                # feeding PE qkv/oproj work between chains keeps it busy)
                for icl in range(4):
                    qkv_unit(0, icl)
                plan = [("q", 1, 0), ("a", 0, 0), ("q", 1, 1), ("a", 0, 1),
                        ("q", 1, 2), ("a", 1, 0), ("q", 1, 3), ("a", 0, 2),
                        ("a", 1, 1), ("a", 0, 3), ("a", 1, 2), ("a", 1, 3)]
                for kind, b, icl in plan:
                    if kind == "q":
                        qkv_unit(b, icl)
                    else:
                        attn_unit(b, icl)
    nc.compile()
    return nc


_PROGS = {}


def _get_prog(causal: bool, reps: int = 1):
    key = (causal, reps)
    if key not in _PROGS:
        _PROGS[key] = _build(causal, reps)
    return _PROGS[key]


def _prep_inputs(x, Wqkv, bqkv, Wo):
    """Per-core input maps (host-side sharding)."""
    x = np.asarray(x, dtype=np.float32)
    Wqkv = np.asarray(Wqkv, dtype=np.float32)
    bqkv = np.asarray(bqkv, dtype=np.float32)
    Wo = np.asarray(Wo, dtype=np.float32)

    # x^T pre-tiled: xt2[p, ic, ct, col] = x^T[128*ct + p, 512*ic + col]
    # -> every i-chunk's load is contiguous per partition
    xt = np.ascontiguousarray(x.reshape(R, C).T).astype(np.float16)  # [C, R]
    xt2 = np.ascontiguousarray(
        xt.reshape(8, 128, 8, IC_W).transpose(1, 2, 0, 3)
    ).reshape(128, 8 * 8 * IC_W)

    # causal mask for the residual 128x128 triangle of a diagonal j-tile,
    # plus a [128, 64] ones block, packed as one [128, 192] constant
    jl = np.arange(JT_W)[:, None]
    il = np.arange(JT_W)[None, :]
    msk = (jl <= il).astype(np.float16)
    mskones = np.concatenate(
        [msk, np.ones((128, 64), dtype=np.float16)], axis=1)

    in_maps = []
    scale = 1.0 / np.sqrt(np.float32(HS))
    for m in range(N_CORES):
        h0, h1 = HPC * m, HPC * m + 1
        cols = {}
        for name, off, sc in (("q", 0, scale), ("k", HS, 1.0), ("v", 2 * HS, 1.0)):
            blk = [Wqkv[:, 192 * h + off:192 * h + off + HS] * sc
                   for h in (h0, h1)]
            bb = [bqkv[192 * h + off:192 * h + off + HS] * sc for h in (h0, h1)]
            cols[name] = (np.concatenate(blk, axis=1),
                          np.concatenate(bb))
        w3 = np.concatenate([cols["q"][0], cols["k"][0], cols["v"][0]], axis=1)
        # pre-tile like xt: w3t[p, ct*384 + col] = w3[128*ct + p, col]
        w3t = np.ascontiguousarray(
            w3.astype(np.float16).reshape(8, 128, 384).transpose(1, 0, 2)
        ).reshape(128, 8 * 384)
        b3 = np.stack([cols["q"][1], cols["k"][1], cols["v"][1]], axis=1)
        wo = Wo[128 * m:128 * (m + 1), :]
        in_maps.append({
            "xt": xt2,
            "w3": w3t,
            "b3": np.ascontiguousarray(b3.astype(np.float32)),  # [128, 3]
            "wo": np.ascontiguousarray(wo).astype(np.float16),
            "mskones": mskones,
        })
    return in_maps


class _Runner:
    """Cached shard_map runner for the SPMD NEFF (avoids re-jit per call)."""

    def __init__(self, nc, donate=True):
        import jax
        from jax.sharding import Mesh, PartitionSpec
        from jax.experimental.shard_map import shard_map
        from concourse import bass2jax

        bass2jax.install_neuronx_cc_hook()

        part_name = (nc.partition_id_tensor.name
                     if nc.partition_id_tensor else None)
        in_names, out_names, out_avals, zero_outs = [], [], [], []
        for alloc in nc.m.functions[0].allocations:
            if not isinstance(alloc, mybir.MemoryLocationSet):
                continue
            name = alloc.memorylocations[0].name
            if alloc.kind == "ExternalInput":
                if name != part_name:
                    in_names.append(name)
            elif alloc.kind == "ExternalOutput":
                out_names.append(name)
                shape = tuple(alloc.tensor_shape)
                dtype = mybir.dt.np(alloc.dtype)
                out_avals.append(jax.core.ShapedArray(shape, dtype))
                zero_outs.append(np.zeros(shape, dtype))
        self.in_names, self.out_names = in_names, out_names
        self.zero_outs = zero_outs
        n_params, n_outs = len(in_names), len(out_names)
        all_in_names = tuple(in_names) + tuple(out_names)
        if part_name is not None:
            all_in_names = all_in_names + (part_name,)

        def _exec(args, outs):
            operands = list(args) + list(outs)
            if part_name is not None:
                operands.append(bass2jax.partition_id_tensor())
            return bass2jax._bass_exec_p.bind(
                *operands,
                out_avals=tuple(out_avals),
                in_names=all_in_names,
                out_names=tuple(out_names),
                lowering_input_output_aliases=(),
                sim_require_finite=True,
                sim_require_nnan=True,
                nc=nc)

        def _body(*args):
            ins, outs = args[:n_params], list(args[n_params:])
            return tuple(_exec(ins, outs))

        devices = jax.devices()[:N_CORES]
        mesh = Mesh(np.asarray(devices), ("core",))
        donate_kw = {}
        if donate:
            donate_kw["donate_argnums"] = tuple(
                range(n_params, n_params + n_outs))
        self._fn = jax.jit(
            shard_map(_body, mesh=mesh,
                      in_specs=(PartitionSpec("core"),) * (n_params + n_outs),
                      out_specs=(PartitionSpec("core"),) * n_outs,
                      check_rep=False),
            keep_unused=True, **donate_kw)

    def __call__(self, in_maps):
        concat_in = [
            np.concatenate([in_maps[c][k] for c in range(N_CORES)], axis=0)
            for k in self.in_names]
        concat_zero = [
            np.zeros((N_CORES * z.shape[0], *z.shape[1:]), z.dtype)
            for z in self.zero_outs]
        out = self._fn(*concat_in, *concat_zero)
        return [
            {k: np.asarray(out[i]).reshape(N_CORES, *self.zero_outs[i].shape)[c]
             for i, k in enumerate(self.out_names)}
            for c in range(N_CORES)]


_RUNNERS = {}


def _get_runner(causal: bool, reps: int = 1, donate: bool = True):
    key = (causal, reps, donate)
    if key not in _RUNNERS:
        _RUNNERS[key] = _Runner(_get_prog(causal, reps), donate=donate)
    return _RUNNERS[key]


def kernel(x, Wqkv, bqkv, Wo, bo, mask):
    causal = bool(np.asarray(mask).item()) if not isinstance(mask, (int, bool)) else bool(mask)
    runner = _get_runner(causal)
    in_maps = _prep_inputs(x, Wqkv, bqkv, Wo)
    results = runner(in_maps)
    acc = np.zeros((R, C), dtype=np.float32)
    for m in range(N_CORES):
        # invert the [8ic, 128p, 4it*1024c] output tiling
        part = results[m]["part"].reshape(8, 128, 4, C) \
            .transpose(0, 2, 1, 3).reshape(R, C)
        acc += part.astype(np.float32)
    acc += np.asarray(bo, dtype=np.float32)[None, :]
    return acc.reshape(B, T, C)
